# revision 4
# baseline (speedup 1.0000x reference)
"""Distributed kNN retrieval kernel for Trainium2 (8 NeuronCores).

Computes, for query batch B=256 against three memory banks of N=131072 rows
(D=512): combined = (0.4*cos(q,Mq) + 0.4*cos(q,Mr) + 0.2*cos(q,Mt)) * strength,
masked below 0.3 to -1.0, then top-5 values + indices per query row
(ties broken by the lowest index, matching jax.lax.top_k).

The three cosine similarities collapse into a single effective fp8 memory
bank E (host-side folding of norms, weights and strengths); each core
matmuls its 16384-row shard against the normalized query with fp8
DoubleRow PE matmuls (K=256 per instruction, 512-wide PSUM-bank outputs).

Reduction pipeline (the bottleneck — engineered around measured rates):
  - PSUM is a 2-deep rotation of [128, 2048] regions (4 banks each): group
    g (4 chunks x 512 rows) x half h (128 queries) fills one region with 8
    matmuls (kh-outer so 4 consecutive matmuls share the stationary query).
  - Each region is drained ONCE as a 2048-wide op (big ops amortize the
    ~0.5-0.8us per-op overhead): a plain Copy of the f32 score's high 16
    bits (bf16) into the odd-u16 slots of a pre-coded [128, 4096] window
    buffer whose even-u16 slots hold a descending index code (4095 - j).
    No relu/threshold on device: raw scores pack fine (negative packed
    words sort below all positives and sub-0.3 candidates are discarded in
    the host merge anyway).  Drains are statically assigned per-region to
    ACT / DVE / DMA-queue (DRAIN_ASSIGN) to keep every unit under the DVE
    MAX8 critical path.  DMA drains copy the strided hi-u16s (bf16
    truncation, ulp-level difference) at zero engine cost.
  - Each full window ([128, 4096] f32 packed = 2 drains) is reduced by a
    single DVE MAX8 into packed top-8 (value AND index); the last window
    per half is split into two 2048 MAX8s to shorten the serial tail.
Host glue decodes 8 cores x 10 units x 8 packed candidates and reduces to
the global top-5 (value desc, index asc) — the standard distributed-kNN
merge.
"""

import sys

if "/opt/trn_rl_repo" not in sys.path:
    sys.path.insert(0, "/opt/trn_rl_repo")

import numpy as np

B = 256
D = 512
N_CORES = 8
CH = 512          # memory rows per chunk (one PSUM bank of output)
GCH = 4           # chunks per group (one 2048-wide drain per group-half)
GW = GCH * CH     # group width in scores (2048)
WIN = 2 * GW      # MAX8 window width (4096)
K_OUT = 5
THRESH = 0.3
EPS = 1e-8
WEIGHTS = (0.4, 0.4, 0.2)
N_WARM = 10       # dummy matmuls to lift the PE HAM throttle during DMA ramp

SCALE_E = 256.0   # fp8 pre-scales: keep elements in the e4m3 normal range
SCALE_Q = 64.0
SCALE = SCALE_E * SCALE_Q

# Drain engine per (group, half) index 2*g+h, g in 0..7:
#   "A" = ACT activation Copy (2.53us/op measured)
#   "V" = DVE tensor_scalar  (2.46us/op, steals MAX8 time)
#   "D" = DMA strided u16 copy on a spare queue (no engine cost, unmeasured)
DRAIN_ASSIGN = list("AAAAAAAAAAAAAAAA")

_cache = {}


def _build(ns, drain_assign=None, split_waits=True):
    """Build the per-core Bass program for a shard of ns memory rows."""
    import concourse.bass as bass
    import concourse.mybir as mybir
    from concourse.tile import TileContext
    from contextlib import ExitStack

    f32 = mybir.dt.float32
    bf16 = mybir.dt.bfloat16
    u16 = mybir.dt.uint16
    u32 = mybir.dt.uint32
    fp8 = mybir.dt.float8e4
    Act = mybir.ActivationFunctionType
    DR = mybir.MatmulPerfMode.DoubleRow

    if drain_assign is None:
        drain_assign = DRAIN_ASSIGN

    n_chunks = ns // CH            # 32
    n_groups = n_chunks // GCH     # 8
    n_wins = n_groups // 2         # 4 per half
    # units per half: (n_wins - 1) full windows + 2 half-window tail units
    n_units = n_wins + 1           # 5

    nc = bass.Bass(trn_type="TRN2")

    q_d = nc.dram_tensor("q", [128, 2, 4, 128], fp8, kind="ExternalInput")
    e_d = nc.dram_tensor("e", [n_chunks * 128, 4, CH], fp8,
                         kind="ExternalInput")
    codes_d = nc.dram_tensor("codes", [128, WIN], u32, kind="ExternalInput")
    vals_d = nc.dram_tensor("vals8", [B, n_units * 8], f32,
                            kind="ExternalOutput")

    q_ap = q_d.ap()
    e_ap = e_d.ap()
    vals_ap = vals_d.ap()

    with TileContext(nc) as tc, ExitStack() as ctx:
        consts = ctx.enter_context(tc.tile_pool(name="consts", bufs=1))
        mpool = ctx.enter_context(tc.tile_pool(name="mpool", bufs=8))
        psum = ctx.enter_context(tc.tile_pool(name="psum", bufs=2,
                                              space="PSUM"))

        # PE pre-warm while the first DMAs land (keeps the HAM clock ramp
        # going before the real matmuls start).
        scratch = consts.tile([128, CH], fp8)
        nc.vector.memset(scratch, 0.0)
        ps_warm = psum.tile([128, GW], f32, tag="S")
        for _ in range(N_WARM):
            nc.tensor.matmul(ps_warm[:, :CH], scratch[:, :128],
                             scratch, start=True, stop=True)

        # Pre-normalized, pre-transposed query: [d_in_block, half, kblk, b].
        qT = consts.tile([128, 2, 4, 128], fp8)
        nc.sync.dma_start(qT, q_ap)

        # Window buffers: [128, WIN] f32, double-buffered per half.  Even
        # u16 of each word = index code (4095 - j), loaded once; odd u16 =
        # bf16 score, rewritten by every drain pass.
        r00 = consts.tile([128, WIN], f32, tag="r00")
        r01 = consts.tile([128, WIN], f32, tag="r01")
        r10 = consts.tile([128, WIN], f32, tag="r10")
        r11 = consts.tile([128, WIN], f32, tag="r11")
        rbuf = [[r00, r01], [r10, r11]]  # [half][win % 2]
        nc.sync.dma_start(r00[:, :].bitcast(u32), codes_d.ap())
        # Fan the codes out with SBUF->SBUF DMAs (zero engine cost).
        for t in (r01, r10, r11):
            nc.sync.dma_start(t[:, :].bitcast(u32), r00[:, :].bitcast(u32))

        # Packed top-8 per unit per half, accumulated then shipped once.
        pc0 = consts.tile([128, n_units * 8], f32, tag="pc0")
        pc1 = consts.tile([128, n_units * 8], f32, tag="pc1")
        pcand = [pc0, pc1]

        for g in range(n_groups):
            ets = []
            for ci in range(GCH):
                c = g * GCH + ci
                et = mpool.tile([128, 4, CH], fp8, tag=f"et{ci}")
                nc.sync.dma_start(et, e_ap[c * 128:(c + 1) * 128])
                ets.append(et)

            w = g // 2           # window index within half
            gq = g % 2           # group slot within the window
            for half in range(2):
                ps = psum.tile([128, GW], f32, tag="S")
                # kh-outer: 4 consecutive matmuls share the stationary
                # query slice.
                for kh in range(2):
                    for ci in range(GCH):
                        nc.tensor.matmul(
                            ps[:, ci * CH:(ci + 1) * CH],
                            qT[:, half, 2 * kh:2 * kh + 2, :],
                            ets[ci][:, 2 * kh:2 * kh + 2, :],
                            start=(kh == 0), stop=(kh == 1),
                            perf_mode=DR,
                        )

                # One 2048-wide drain of the raw scores' high u16 (bf16)
                # into the odd-u16 slots of the window buffer.
                rb = rbuf[half][w % 2]
                eng = drain_assign[2 * g + half]
                lo = gq * GW
                out_slots = rb[:, lo:lo + GW].bitcast(bf16).rearrange(
                    "p (j two) -> p j two", two=2)[:, :, 1]
                if eng == "A":
                    nc.scalar.activation(out_slots, ps, Act.Copy)
                elif eng == "V":
                    nc.vector.tensor_copy(out_slots, ps)
                else:  # "D": strided u16 DMA copy (bf16 truncation)
                    src16 = ps[:, :].bitcast(u16).rearrange(
                        "p (j two) -> p j two", two=2)[:, :, 1]
                    dst16 = rb[:, lo:lo + GW].bitcast(u16).rearrange(
                        "p (j two) -> p j two", two=2)[:, :, 1]
                    nc.sync.dma_start(dst16, src16)

                last_win = (w == n_wins - 1)
                if last_win:
                    # Tail: two half-window MAX8 units, each fired as soon
                    # as its group is drained.
                    u = n_wins - 1 + gq
                    nc.vector.max(
                        out=pcand[half][:, u * 8:(u + 1) * 8],
                        in_=rb[:, lo:lo + GW])
                elif gq == 1:
                    nc.vector.max(
                        out=pcand[half][:, w * 8:(w + 1) * 8],
                        in_=rb)

        for half in range(2):
            nc.sync.dma_start(
                vals_ap[half * 128:(half + 1) * 128, :], pcand[half])

    if split_waits:
        _split_tsp_waits(nc, mybir)
    return nc


def _split_tsp_waits(nc, mybir):
    """This walrus build rejects ANY instruction carrying more than one
    sync-wait command in its encoding. Hoist excess waits onto same-engine
    NoOps inserted just before — engines execute their stream in order, so
    gating the NoOp gates the op."""
    skip = {"NoOp"}
    fn = nc.m.functions[0]
    for blk in fn.blocks:
        insts = list(blk.instructions)
        new_insts = []
        changed = False
        for ins in insts:
            si = ins.sync_info
            waits = list(si.on_wait) if si is not None and si.on_wait else []
            if ins.opcode not in skip and len(waits) > 1:
                for wi, w in enumerate(waits[:-1]):
                    new_insts.append(mybir.InstNoOp(
                        name=f"{ins.name}-wn{wi}",
                        engine=ins.engine,
                        sync_info=mybir.SyncInfo(on_wait=[w], on_update=[]),
                    ))
                ins.sync_info = mybir.SyncInfo(
                    on_wait=waits[-1:],
                    on_update=list(si.on_update) if si.on_update else [],
                )
                changed = True
            new_insts.append(ins)
        if changed:
            blk.instructions = new_insts


def _get_program(ns):
    if ns not in _cache:
        _cache[ns] = _build(ns)
    return _cache[ns]


def make_in_maps(query, mem_questions, mem_responses, mem_traces, mem_strengths):
    """Host-side sharding: fold the per-row normalization, bank weights and
    strengths into one effective fp8 memory bank, pre-transposed into
    matmul layout; normalize + transpose the query."""
    import ml_dtypes

    q = np.asarray(query, dtype=np.float32)
    s = np.asarray(mem_strengths, dtype=np.float32)

    mdt = ml_dtypes.float8_e4m3

    qh = q / (np.linalg.norm(q, axis=1, keepdims=True) + EPS)
    # [p, half, kb, b] = qh[half*128 + b, kb*128 + p]
    qT = np.ascontiguousarray(
        qh.reshape(2, 128, 4, 128).transpose(3, 0, 2, 1) * SCALE_Q
    ).astype(mdt)

    e = None
    for w, m in zip(WEIGHTS,
                    (mem_questions, mem_responses, mem_traces)):
        m = np.asarray(m, dtype=np.float32)
        f = (w / (np.sqrt(np.einsum('nd,nd->n', m, m)) + EPS)).astype(
            np.float32)
        t = m * f[:, None]
        e = t if e is None else e + t
    e *= s[:, None] * SCALE_E
    np.clip(e, -240.0, 240.0, out=e)
    e16 = e.astype(mdt)

    codes = np.broadcast_to(
        np.arange(WIN - 1, -1, -1, dtype=np.uint32)[None, :], (128, WIN)
    ).copy()

    n = e16.shape[0]
    ns = n // N_CORES
    n_chunks = ns // CH
    in_maps = []
    for c in range(N_CORES):
        ec = e16[c * ns:(c + 1) * ns]
        # [chunk*128 + p, kb, n] = ec[chunk*CH + n, kb*128 + p]
        ed = np.ascontiguousarray(
            ec.reshape(n_chunks, CH, 4, 128).transpose(0, 3, 2, 1)
        ).reshape(n_chunks * 128, 4, CH)
        in_maps.append({"q": qT, "e": ed, "codes": codes})
    return in_maps, ns


def merge_candidates(per_core, ns, k):
    """Decode the packed (bf16 raw score | window-local index code)
    candidates of all cores and units, apply the 0.3 threshold mask, and
    reduce to the global top-k (value desc, index asc) — matching
    jax.lax.top_k on the masked array.

    Exactness: every score above the threshold that can enter the global
    top-5 is within its window's top-8 (a window contributes at most 5),
    so the survivor set is complete; -1 fills use the smallest free
    indices, matching top_k's tie-break on the all(-1) masked tail."""
    import ml_dtypes

    inv = 1.0 / SCALE
    n_wins = ns // WIN                      # 4 per half
    n_units = n_wins + 1                    # last window split in two
    # unit -> window base within a half's 16384 rows; the two tail units
    # share the last window's base (codes stay window-local)
    unit_base = np.array([w * WIN for w in range(n_wins - 1)]
                         + [(n_wins - 1) * WIN, (n_wins - 1) * WIN])
    packed = np.concatenate(
        [np.ascontiguousarray(np.asarray(r["vals8"], dtype=np.float32))
         for r in per_core], axis=1)        # [B, n_cores * n_units * 8]
    bits = packed.view(np.uint32)
    cand_vals = (bits >> 16).astype(np.uint16).view(
        ml_dtypes.bfloat16).astype(np.float32) * inv
    # window-local position; codes are window-relative (4095 - j) even for
    # the split tail units
    j_local = (WIN - 1) - (bits & 0xFFFF).astype(np.int64)
    base = np.tile(np.repeat(unit_base, 8)[None, :], (1, len(per_core)))
    core = np.repeat(np.arange(len(per_core)), n_units * 8)[None, :]
    cand_idx = core * ns + base + j_local

    surv = cand_vals > THRESH
    masked_vals = np.where(surv, cand_vals, -np.inf)
    order1 = np.argsort(cand_idx, axis=1, kind="stable")
    v1 = np.take_along_axis(masked_vals, order1, axis=1)
    i1 = np.take_along_axis(cand_idx, order1, axis=1)
    order2 = np.argsort(-v1, axis=1, kind="stable")
    vals = np.take_along_axis(v1, order2, axis=1)[:, :k].copy()
    idx = np.take_along_axis(i1, order2, axis=1)[:, :k].copy()
    nrows = vals.shape[0]
    for r in range(nrows):
        m = int((vals[r] > -np.inf).sum())
        if m >= k:
            continue
        taken = set(int(x) for x in idx[r, :m])
        fill = []
        cand = 0
        while len(fill) < k - m:
            if cand not in taken:
                fill.append(cand)
            cand += 1
        vals[r, m:] = -1.0
        idx[r, m:] = fill
    return vals.astype(np.float32), idx.astype(np.int32)


def _install_ntff_shim():
    """Register the axon NTFF profile hook (the agent image lacks
    antenv.axon_hooks; recreate it per the documented ctypes C ABI)."""
    import sys as _sys
    import types
    import ctypes
    import contextlib

    if "antenv.axon_hooks" in _sys.modules:
        return
    so_path = "/opt/axon/libaxon_pjrt.so"
    lib = ctypes.CDLL(so_path)
    if not hasattr(lib, "axon_start_nrt_profile"):
        return
    lib.axon_start_nrt_profile.argtypes = [
        ctypes.POINTER(ctypes.c_int64), ctypes.c_size_t]
    lib.axon_start_nrt_profile.restype = ctypes.c_int64
    lib.axon_stop_nrt_profile.argtypes = [ctypes.c_char_p]
    lib.axon_stop_nrt_profile.restype = ctypes.c_int64

    @contextlib.contextmanager
    def _hook(output_dir, device_ids):
        import jax
        jax.devices()
        if device_ids:
            ids = (ctypes.c_int64 * len(device_ids))(*device_ids)
            rc = lib.axon_start_nrt_profile(ids, len(device_ids))
        else:
            rc = lib.axon_start_nrt_profile(None, 0)
        if rc != 0:
            raise RuntimeError(f"axon_start_nrt_profile rc={rc}")
        try:
            yield
        finally:
            n = lib.axon_stop_nrt_profile(str(output_dir).encode())
            print(f"ntff profile: {n} file(s) written to {output_dir}",
                  file=_sys.stderr)

    mod = types.ModuleType("antenv.axon_hooks")
    mod._hook = _hook
    mod.get_axon_ntff_profile_hook = lambda: _hook
    mod.set_axon_ntff_profile_hook = lambda h: None
    _sys.modules["antenv.axon_hooks"] = mod


def kernel(query, mem_questions, mem_responses, mem_traces, mem_strengths,
           top_k, _trace=False, _results_box=None):
    from concourse import bass_utils

    if _trace:
        _install_ntff_shim()

    k = int(top_k)
    in_maps, ns = make_in_maps(
        query, mem_questions, mem_responses, mem_traces, mem_strengths)
    nc = _get_program(ns)
    res = bass_utils.run_bass_kernel_spmd(
        nc, in_maps, core_ids=list(range(N_CORES)), trace=_trace)
    if _results_box is not None:
        _results_box.append(res)
    return merge_candidates(res.results, ns, k)


# revision 12
# speedup vs baseline: 1.3571x; 1.3571x over previous
"""Distributed kNN retrieval kernel for Trainium2 (8 NeuronCores).

Computes, for query batch B=256 against three memory banks of N=131072 rows
(D=512): combined = (0.4*cos(q,Mq) + 0.4*cos(q,Mr) + 0.2*cos(q,Mt)) * strength,
masked below 0.3 to -1.0, then top-5 values + indices per query row
(ties broken by the lowest index, matching jax.lax.top_k).

The three cosine similarities collapse into a single effective fp8 memory
bank E (host-side folding of norms, weights and strengths); each core
matmuls its 16384-row shard against the normalized query with fp8
DoubleRow PE matmuls (K=256 per instruction, 512-wide PSUM-bank outputs).

Reduction pipeline (the bottleneck — engineered around measured rates):
  - PSUM is a 2-deep rotation of [128, 2048] regions (4 banks each): group
    g (4 chunks x 512 rows) x half h (128 queries) fills one region with 8
    matmuls (kh-outer so 4 consecutive matmuls share the stationary query).
  - Each region is drained ONCE as a 2048-wide op (big ops amortize the
    ~0.5-0.8us per-op overhead): a plain Copy of the f32 score's high 16
    bits (bf16) into the odd-u16 slots of a pre-coded [128, 4096] window
    buffer whose even-u16 slots hold a descending index code (4095 - j).
    No relu/threshold on device: raw scores pack fine (negative packed
    words sort below all positives and sub-0.3 candidates are discarded in
    the host merge anyway).  Drains are statically assigned per-region to
    ACT / DVE / DMA-queue (DRAIN_ASSIGN) to keep every unit under the DVE
    MAX8 critical path.  DMA drains copy the strided hi-u16s (bf16
    truncation, ulp-level difference) at zero engine cost.
  - Each full window ([128, 4096] f32 packed = 2 drains) is reduced by a
    single DVE MAX8 into packed top-8 (value AND index); the last window
    per half is split into two 2048 MAX8s to shorten the serial tail.
Host glue decodes 8 cores x 10 units x 8 packed candidates and reduces to
the global top-5 (value desc, index asc) — the standard distributed-kNN
merge.
"""

import sys

if "/opt/trn_rl_repo" not in sys.path:
    sys.path.insert(0, "/opt/trn_rl_repo")

import numpy as np

B = 256
D = 512
N_CORES = 8
CH = 512          # memory rows per chunk (one PSUM bank of output)
GCH = 4           # chunks per group (one 2048-wide drain per group-half)
GW = GCH * CH     # group width in scores (2048)
WIN = GW          # MAX8 window width == drain width (2048)
K_OUT = 5
THRESH = 0.3
EPS = 1e-8
WEIGHTS = (0.4, 0.4, 0.2)
N_WARM = 10       # dummy matmuls to lift the PE HAM throttle during DMA ramp

SCALE_E = 256.0   # fp8 pre-scales: keep elements in the e4m3 normal range
SCALE_Q = 64.0
SCALE = SCALE_E * SCALE_Q

# Drain engine per (group, half) index 2*g+h, g in 0..7:
#   "A" = ACT activation Copy (~2.59us/op measured)
#   "V" = DVE tensor_copy    (~2.46us/op, steals MAX8 time)
# ACT alone is 16 x 2.59 = 41.4us serial vs DVE's 16 MAX8 = 37.3us; one
# mid-stream DVE drain balances the two at ~39us each.
DRAIN_ASSIGN = list("AAAAAAAAAAAVAAAA")

_cache = {}


def _build(ns, drain_assign=None, split_waits=True):
    """Build the per-core Bass program for a shard of ns memory rows."""
    import concourse.bass as bass
    import concourse.mybir as mybir
    from concourse.tile import TileContext
    from contextlib import ExitStack

    f32 = mybir.dt.float32
    bf16 = mybir.dt.bfloat16
    u16 = mybir.dt.uint16
    u32 = mybir.dt.uint32
    fp8 = mybir.dt.float8e4
    Act = mybir.ActivationFunctionType
    DR = mybir.MatmulPerfMode.DoubleRow

    if drain_assign is None:
        drain_assign = DRAIN_ASSIGN

    n_chunks = ns // CH            # 32
    n_groups = n_chunks // GCH     # 8
    n_units = n_groups             # one MAX8 unit per (group, half)

    nc = bass.Bass(trn_type="TRN2")

    q_d = nc.dram_tensor("q", [128, 2, 4, 128], fp8, kind="ExternalInput")
    e_d = nc.dram_tensor("e", [n_chunks * 128, 4, CH], fp8,
                         kind="ExternalInput")
    vals_d = nc.dram_tensor("vals8", [B, n_units * 8], f32,
                            kind="ExternalOutput")

    q_ap = q_d.ap()
    e_ap = e_d.ap()
    vals_ap = vals_d.ap()

    with TileContext(nc) as tc, ExitStack() as ctx:
        consts = ctx.enter_context(tc.tile_pool(name="consts", bufs=1))
        mpool = ctx.enter_context(tc.tile_pool(name="mpool", bufs=8))
        psum = ctx.enter_context(tc.tile_pool(name="psum", bufs=2,
                                              space="PSUM"))

        # PE pre-warm while the first DMAs land (keeps the HAM clock ramp
        # going before the real matmuls start).
        scratch = consts.tile([128, CH], fp8)
        nc.vector.memset(scratch, 0.0)
        ps_warm = psum.tile([128, GW], f32, tag="S")
        for _ in range(N_WARM):
            nc.tensor.matmul(ps_warm[:, :CH], scratch[:, :128],
                             scratch, start=True, stop=True)

        # Pre-normalized, pre-transposed query: [d_in_block, half, kblk, b].
        qT = consts.tile([128, 2, 4, 128], fp8)
        nc.sync.dma_start(qT, q_ap)

        # Window buffers: [128, WIN] f32, double-buffered per half.  Even
        # u16 of each word = index code (2047 - j), generated once by Pool
        # iotas (the Pool engine is otherwise idle and needs no DMA, so the
        # codes are ready long before the first drain); odd u16 = bf16
        # score, rewritten by every drain pass.
        r00 = consts.tile([128, WIN], f32, tag="r00")
        r01 = consts.tile([128, WIN], f32, tag="r01")
        r10 = consts.tile([128, WIN], f32, tag="r10")
        r11 = consts.tile([128, WIN], f32, tag="r11")
        rbuf = [[r00, r01], [r10, r11]]  # [half][group % 2]
        # In first-need order: g0h0, g0h1, g1h0, g1h1.
        for t in (r00, r10, r01, r11):
            nc.gpsimd.iota(t[:, :].bitcast(u32), [[-1, WIN]], base=WIN - 1,
                           channel_multiplier=0)

        # Packed top-8 per unit per half, accumulated then shipped once.
        pc0 = consts.tile([128, n_units * 8], f32, tag="pc0")
        pc1 = consts.tile([128, n_units * 8], f32, tag="pc1")
        pcand = [pc0, pc1]

        for g in range(n_groups):
            ets = []
            for ci in range(GCH):
                c = g * GCH + ci
                et = mpool.tile([128, 4, CH], fp8, tag=f"et{ci}")
                nc.sync.dma_start(et, e_ap[c * 128:(c + 1) * 128])
                ets.append(et)

            for half in range(2):
                ps = psum.tile([128, GW], f32, tag="S")
                # kh-outer: 4 consecutive matmuls share the stationary
                # query slice.
                for kh in range(2):
                    for ci in range(GCH):
                        nc.tensor.matmul(
                            ps[:, ci * CH:(ci + 1) * CH],
                            qT[:, half, 2 * kh:2 * kh + 2, :],
                            ets[ci][:, 2 * kh:2 * kh + 2, :],
                            start=(kh == 0), stop=(kh == 1),
                            perf_mode=DR,
                        )

                # One 2048-wide drain of the raw scores' high u16 (bf16)
                # into the odd-u16 slots of the window buffer.
                rb = rbuf[half][g % 2]
                eng = drain_assign[2 * g + half]
                out_slots = rb[:, :].bitcast(bf16).rearrange(
                    "p (j two) -> p j two", two=2)[:, :, 1]
                if eng == "A":
                    nc.scalar.activation(out_slots, ps, Act.Copy)
                else:
                    nc.vector.tensor_copy(out_slots, ps)

                nc.vector.max(
                    out=pcand[half][:, g * 8:(g + 1) * 8], in_=rb)

        for half in range(2):
            nc.sync.dma_start(
                vals_ap[half * 128:(half + 1) * 128, :], pcand[half])

    if split_waits:
        _split_tsp_waits(nc, mybir)
    return nc


def _split_tsp_waits(nc, mybir):
    """This walrus build rejects ANY instruction carrying more than one
    sync-wait command in its encoding. Hoist excess waits onto same-engine
    NoOps inserted just before — engines execute their stream in order, so
    gating the NoOp gates the op."""
    skip = {"NoOp"}
    fn = nc.m.functions[0]
    for blk in fn.blocks:
        insts = list(blk.instructions)
        new_insts = []
        changed = False
        for ins in insts:
            si = ins.sync_info
            waits = list(si.on_wait) if si is not None and si.on_wait else []
            if ins.opcode not in skip and len(waits) > 1:
                for wi, w in enumerate(waits[:-1]):
                    new_insts.append(mybir.InstNoOp(
                        name=f"{ins.name}-wn{wi}",
                        engine=ins.engine,
                        sync_info=mybir.SyncInfo(on_wait=[w], on_update=[]),
                    ))
                ins.sync_info = mybir.SyncInfo(
                    on_wait=waits[-1:],
                    on_update=list(si.on_update) if si.on_update else [],
                )
                changed = True
            new_insts.append(ins)
        if changed:
            blk.instructions = new_insts


def _get_program(ns):
    if ns not in _cache:
        _cache[ns] = _build(ns)
    return _cache[ns]


def make_in_maps(query, mem_questions, mem_responses, mem_traces, mem_strengths):
    """Host-side sharding: fold the per-row normalization, bank weights and
    strengths into one effective fp8 memory bank, pre-transposed into
    matmul layout; normalize + transpose the query."""
    import ml_dtypes

    q = np.asarray(query, dtype=np.float32)
    s = np.asarray(mem_strengths, dtype=np.float32)

    mdt = ml_dtypes.float8_e4m3

    qh = q / (np.linalg.norm(q, axis=1, keepdims=True) + EPS)
    # [p, half, kb, b] = qh[half*128 + b, kb*128 + p]
    qT = np.ascontiguousarray(
        qh.reshape(2, 128, 4, 128).transpose(3, 0, 2, 1) * SCALE_Q
    ).astype(mdt)

    e = None
    for w, m in zip(WEIGHTS,
                    (mem_questions, mem_responses, mem_traces)):
        m = np.asarray(m, dtype=np.float32)
        f = (w / (np.sqrt(np.einsum('nd,nd->n', m, m)) + EPS)).astype(
            np.float32)
        t = m * f[:, None]
        e = t if e is None else e + t
    e *= s[:, None] * SCALE_E
    np.clip(e, -240.0, 240.0, out=e)
    e16 = e.astype(mdt)

    n = e16.shape[0]
    ns = n // N_CORES
    n_chunks = ns // CH
    in_maps = []
    for c in range(N_CORES):
        ec = e16[c * ns:(c + 1) * ns]
        # [chunk*128 + p, kb, n] = ec[chunk*CH + n, kb*128 + p]
        ed = np.ascontiguousarray(
            ec.reshape(n_chunks, CH, 4, 128).transpose(0, 3, 2, 1)
        ).reshape(n_chunks * 128, 4, CH)
        in_maps.append({"q": qT, "e": ed})
    return in_maps, ns


def merge_candidates(per_core, ns, k):
    """Decode the packed (bf16 raw score | window-local index code)
    candidates of all cores and units, apply the 0.3 threshold mask, and
    reduce to the global top-k (value desc, index asc) — matching
    jax.lax.top_k on the masked array.

    Exactness: every score above the threshold that can enter the global
    top-5 is within its window's top-8 (a window contributes at most 5),
    so the survivor set is complete; -1 fills use the smallest free
    indices, matching top_k's tie-break on the all(-1) masked tail."""
    import ml_dtypes

    inv = 1.0 / SCALE
    n_units = ns // WIN                     # 8 windows per half
    unit_base = np.arange(n_units) * WIN
    packed = np.concatenate(
        [np.ascontiguousarray(np.asarray(r["vals8"], dtype=np.float32))
         for r in per_core], axis=1)        # [B, n_cores * n_units * 8]
    bits = packed.view(np.uint32)
    cand_vals = (bits >> 16).astype(np.uint16).view(
        ml_dtypes.bfloat16).astype(np.float32) * inv
    # window-local position; codes are window-relative (4095 - j) even for
    # the split tail units
    j_local = (WIN - 1) - (bits & 0xFFFF).astype(np.int64)
    base = np.tile(np.repeat(unit_base, 8)[None, :], (1, len(per_core)))
    core = np.repeat(np.arange(len(per_core)), n_units * 8)[None, :]
    cand_idx = core * ns + base + j_local

    surv = cand_vals > THRESH
    masked_vals = np.where(surv, cand_vals, -np.inf)
    order1 = np.argsort(cand_idx, axis=1, kind="stable")
    v1 = np.take_along_axis(masked_vals, order1, axis=1)
    i1 = np.take_along_axis(cand_idx, order1, axis=1)
    order2 = np.argsort(-v1, axis=1, kind="stable")
    vals = np.take_along_axis(v1, order2, axis=1)[:, :k].copy()
    idx = np.take_along_axis(i1, order2, axis=1)[:, :k].copy()
    nrows = vals.shape[0]
    for r in range(nrows):
        m = int((vals[r] > -np.inf).sum())
        if m >= k:
            continue
        taken = set(int(x) for x in idx[r, :m])
        fill = []
        cand = 0
        while len(fill) < k - m:
            if cand not in taken:
                fill.append(cand)
            cand += 1
        vals[r, m:] = -1.0
        idx[r, m:] = fill
    return vals.astype(np.float32), idx.astype(np.int32)


def _install_ntff_shim():
    """Register the axon NTFF profile hook (the agent image lacks
    antenv.axon_hooks; recreate it per the documented ctypes C ABI)."""
    import sys as _sys
    import types
    import ctypes
    import contextlib

    if "antenv.axon_hooks" in _sys.modules:
        return
    so_path = "/opt/axon/libaxon_pjrt.so"
    lib = ctypes.CDLL(so_path)
    if not hasattr(lib, "axon_start_nrt_profile"):
        return
    lib.axon_start_nrt_profile.argtypes = [
        ctypes.POINTER(ctypes.c_int64), ctypes.c_size_t]
    lib.axon_start_nrt_profile.restype = ctypes.c_int64
    lib.axon_stop_nrt_profile.argtypes = [ctypes.c_char_p]
    lib.axon_stop_nrt_profile.restype = ctypes.c_int64

    @contextlib.contextmanager
    def _hook(output_dir, device_ids):
        import jax
        jax.devices()
        if device_ids:
            ids = (ctypes.c_int64 * len(device_ids))(*device_ids)
            rc = lib.axon_start_nrt_profile(ids, len(device_ids))
        else:
            rc = lib.axon_start_nrt_profile(None, 0)
        if rc != 0:
            raise RuntimeError(f"axon_start_nrt_profile rc={rc}")
        try:
            yield
        finally:
            n = lib.axon_stop_nrt_profile(str(output_dir).encode())
            print(f"ntff profile: {n} file(s) written to {output_dir}",
                  file=_sys.stderr)

    mod = types.ModuleType("antenv.axon_hooks")
    mod._hook = _hook
    mod.get_axon_ntff_profile_hook = lambda: _hook
    mod.set_axon_ntff_profile_hook = lambda h: None
    _sys.modules["antenv.axon_hooks"] = mod


def kernel(query, mem_questions, mem_responses, mem_traces, mem_strengths,
           top_k, _trace=False, _results_box=None):
    from concourse import bass_utils

    if _trace:
        _install_ntff_shim()

    k = int(top_k)
    in_maps, ns = make_in_maps(
        query, mem_questions, mem_responses, mem_traces, mem_strengths)
    nc = _get_program(ns)
    res = bass_utils.run_bass_kernel_spmd(
        nc, in_maps, core_ids=list(range(N_CORES)), trace=_trace)
    if _results_box is not None:
        _results_box.append(res)
    return merge_candidates(res.results, ns, k)


# revision 13
# speedup vs baseline: 1.5271x; 1.1253x over previous
"""Distributed kNN retrieval kernel for Trainium2 (8 NeuronCores).

Computes, for query batch B=256 against three memory banks of N=131072 rows
(D=512): combined = (0.4*cos(q,Mq) + 0.4*cos(q,Mr) + 0.2*cos(q,Mt)) * strength,
masked below 0.3 to -1.0, then top-5 values + indices per query row
(ties broken by the lowest index, matching jax.lax.top_k).

The three cosine similarities collapse into a single effective fp8 memory
bank E (host-side folding of norms, weights and strengths); each core
matmuls its 16384-row shard against the normalized query with fp8
DoubleRow PE matmuls (K=256 per instruction, 512-wide PSUM-bank outputs).

Reduction pipeline (the bottleneck — engineered around measured rates):
  - PSUM is a 2-deep rotation of [128, 2048] regions (4 banks each): group
    g (4 chunks x 512 rows) x half h (128 queries) fills one region with 8
    matmuls (kh-outer so 4 consecutive matmuls share the stationary query).
  - Each region is drained ONCE as a 2048-wide op (big ops amortize the
    ~0.5-0.8us per-op overhead): a plain Copy of the f32 score's high 16
    bits (bf16) into the odd-u16 slots of a pre-coded [128, 4096] window
    buffer whose even-u16 slots hold a descending index code (4095 - j).
    No relu/threshold on device: raw scores pack fine (negative packed
    words sort below all positives and sub-0.3 candidates are discarded in
    the host merge anyway).  Drains are statically assigned per-region to
    ACT / DVE / DMA-queue (DRAIN_ASSIGN) to keep every unit under the DVE
    MAX8 critical path.  DMA drains copy the strided hi-u16s (bf16
    truncation, ulp-level difference) at zero engine cost.
  - Each full window ([128, 4096] f32 packed = 2 drains) is reduced by a
    single DVE MAX8 into packed top-8 (value AND index); the last window
    per half is split into two 2048 MAX8s to shorten the serial tail.
Host glue decodes 8 cores x 10 units x 8 packed candidates and reduces to
the global top-5 (value desc, index asc) — the standard distributed-kNN
merge.
"""

import sys

if "/opt/trn_rl_repo" not in sys.path:
    sys.path.insert(0, "/opt/trn_rl_repo")

import numpy as np

B = 256
D = 512
N_CORES = 8
CH = 512          # memory rows per chunk (one PSUM bank of output)
GCH = 4           # chunks per group (one 2048-wide drain per group-half)
GW = GCH * CH     # group width in scores (2048)
WIN = GW          # MAX8 window width == drain width (2048)
K_OUT = 5
THRESH = 0.3
EPS = 1e-8
WEIGHTS = (0.4, 0.4, 0.2)
N_WARM = 10       # dummy matmuls to lift the PE HAM throttle during DMA ramp

SCALE_E = 256.0   # fp8 pre-scales: keep elements in the e4m3 normal range
SCALE_Q = 64.0
SCALE = SCALE_E * SCALE_Q

# Drain engine per (group, half) index 2*g+h, g in 0..7:
#   "A" = ACT activation Copy (~2.59us/op measured)
#   "V" = DVE tensor_copy    (~2.46us/op, steals MAX8 time)
# ACT alone is 16 x 2.59 = 41.4us serial vs DVE's 16 MAX8 = 37.3us; one
# mid-stream DVE drain balances the two at ~39us each.
DRAIN_ASSIGN = list("AAAAAAAAAAAAAAAA")

_cache = {}


def _build(ns, drain_assign=None, split_waits=True):
    """Build the per-core Bass program for a shard of ns memory rows."""
    import concourse.bass as bass
    import concourse.mybir as mybir
    from concourse.tile import TileContext
    from contextlib import ExitStack

    f32 = mybir.dt.float32
    bf16 = mybir.dt.bfloat16
    u16 = mybir.dt.uint16
    u32 = mybir.dt.uint32
    fp8 = mybir.dt.float8e4
    Act = mybir.ActivationFunctionType
    DR = mybir.MatmulPerfMode.DoubleRow

    if drain_assign is None:
        drain_assign = DRAIN_ASSIGN

    n_chunks = ns // CH            # 32
    n_groups = n_chunks // GCH     # 8
    n_units = n_groups             # one MAX8 unit per (group, half)

    nc = bass.Bass(trn_type="TRN2")

    q_d = nc.dram_tensor("q", [128, 2, 4, 128], fp8, kind="ExternalInput")
    e_d = nc.dram_tensor("e", [n_chunks * 128, 4, CH], fp8,
                         kind="ExternalInput")
    vals_d = nc.dram_tensor("vals8", [B, n_units * 8], f32,
                            kind="ExternalOutput")

    q_ap = q_d.ap()
    e_ap = e_d.ap()
    vals_ap = vals_d.ap()

    with TileContext(nc) as tc, ExitStack() as ctx:
        consts = ctx.enter_context(tc.tile_pool(name="consts", bufs=1))
        mpool = ctx.enter_context(tc.tile_pool(name="mpool", bufs=8))
        psum = ctx.enter_context(tc.tile_pool(name="psum", bufs=2,
                                              space="PSUM"))

        # PE pre-warm while the first DMAs land (keeps the HAM clock ramp
        # going before the real matmuls start).
        scratch = consts.tile([128, CH], fp8)
        nc.vector.memset(scratch, 0.0)
        ps_warm = psum.tile([128, GW], f32, tag="S")
        for _ in range(N_WARM):
            nc.tensor.matmul(ps_warm[:, :CH], scratch[:, :128],
                             scratch, start=True, stop=True)

        # Pre-normalized, pre-transposed query: [d_in_block, half, kblk, b].
        qT = consts.tile([128, 2, 4, 128], fp8)
        nc.sync.dma_start(qT, q_ap)

        # Window buffers: [128, WIN] f32, double-buffered per half.  Even
        # u16 of each word = index code (2047 - j), generated once by Pool
        # iotas (the Pool engine is otherwise idle and needs no DMA, so the
        # codes are ready long before the first drain); odd u16 = bf16
        # score, rewritten by every drain pass.
        r00 = consts.tile([128, WIN], f32, tag="r00")
        r01 = consts.tile([128, WIN], f32, tag="r01")
        r10 = consts.tile([128, WIN], f32, tag="r10")
        r11 = consts.tile([128, WIN], f32, tag="r11")
        rbuf = [[r00, r01], [r10, r11]]  # [half][group % 2]
        # In first-need order: g0h0, g0h1, g1h0, g1h1.
        for t in (r00, r10, r01, r11):
            nc.gpsimd.iota(t[:, :].bitcast(u32), [[-1, WIN]], base=WIN - 1,
                           channel_multiplier=0)

        # Packed top-8 per unit per half, accumulated then shipped once.
        pc0 = consts.tile([128, n_units * 8], f32, tag="pc0")
        pc1 = consts.tile([128, n_units * 8], f32, tag="pc1")
        pcand = [pc0, pc1]

        for g in range(n_groups):
            ets = []
            for ci in range(GCH):
                c = g * GCH + ci
                et = mpool.tile([128, 4, CH], fp8, tag=f"et{ci}")
                nc.sync.dma_start(et, e_ap[c * 128:(c + 1) * 128])
                ets.append(et)

            for half in range(2):
                ps = psum.tile([128, GW], f32, tag="S")
                # kh-outer: 4 consecutive matmuls share the stationary
                # query slice.
                for kh in range(2):
                    for ci in range(GCH):
                        nc.tensor.matmul(
                            ps[:, ci * CH:(ci + 1) * CH],
                            qT[:, half, 2 * kh:2 * kh + 2, :],
                            ets[ci][:, 2 * kh:2 * kh + 2, :],
                            start=(kh == 0), stop=(kh == 1),
                            perf_mode=DR,
                        )

                # One 2048-wide drain of the raw scores' high u16 (bf16)
                # into the odd-u16 slots of the window buffer.
                rb = rbuf[half][g % 2]
                eng = drain_assign[2 * g + half]
                out_slots = rb[:, :].bitcast(bf16).rearrange(
                    "p (j two) -> p j two", two=2)[:, :, 1]
                if eng == "A":
                    nc.scalar.activation(out_slots, ps, Act.Copy)
                else:
                    nc.vector.tensor_copy(out_slots, ps)

                nc.vector.max(
                    out=pcand[half][:, g * 8:(g + 1) * 8], in_=rb)

        for half in range(2):
            nc.sync.dma_start(
                vals_ap[half * 128:(half + 1) * 128, :], pcand[half])

    if split_waits:
        _split_tsp_waits(nc, mybir)
    return nc


def _split_tsp_waits(nc, mybir):
    """This walrus build rejects ANY instruction carrying more than one
    sync-wait command in its encoding. Hoist excess waits onto same-engine
    NoOps inserted just before — engines execute their stream in order, so
    gating the NoOp gates the op."""
    skip = {"NoOp"}
    fn = nc.m.functions[0]
    for blk in fn.blocks:
        insts = list(blk.instructions)
        new_insts = []
        changed = False
        for ins in insts:
            si = ins.sync_info
            waits = list(si.on_wait) if si is not None and si.on_wait else []
            if ins.opcode not in skip and len(waits) > 1:
                for wi, w in enumerate(waits[:-1]):
                    new_insts.append(mybir.InstNoOp(
                        name=f"{ins.name}-wn{wi}",
                        engine=ins.engine,
                        sync_info=mybir.SyncInfo(on_wait=[w], on_update=[]),
                    ))
                ins.sync_info = mybir.SyncInfo(
                    on_wait=waits[-1:],
                    on_update=list(si.on_update) if si.on_update else [],
                )
                changed = True
            new_insts.append(ins)
        if changed:
            blk.instructions = new_insts


def _get_program(ns):
    if ns not in _cache:
        _cache[ns] = _build(ns)
    return _cache[ns]


def make_in_maps(query, mem_questions, mem_responses, mem_traces, mem_strengths):
    """Host-side sharding: fold the per-row normalization, bank weights and
    strengths into one effective fp8 memory bank, pre-transposed into
    matmul layout; normalize + transpose the query."""
    import ml_dtypes

    q = np.asarray(query, dtype=np.float32)
    s = np.asarray(mem_strengths, dtype=np.float32)

    mdt = ml_dtypes.float8_e4m3

    qh = q / (np.linalg.norm(q, axis=1, keepdims=True) + EPS)
    # [p, half, kb, b] = qh[half*128 + b, kb*128 + p]
    qT = np.ascontiguousarray(
        qh.reshape(2, 128, 4, 128).transpose(3, 0, 2, 1) * SCALE_Q
    ).astype(mdt)

    e = None
    for w, m in zip(WEIGHTS,
                    (mem_questions, mem_responses, mem_traces)):
        m = np.asarray(m, dtype=np.float32)
        f = (w / (np.sqrt(np.einsum('nd,nd->n', m, m)) + EPS)).astype(
            np.float32)
        t = m * f[:, None]
        e = t if e is None else e + t
    e *= s[:, None] * SCALE_E
    np.clip(e, -240.0, 240.0, out=e)
    e16 = e.astype(mdt)

    n = e16.shape[0]
    ns = n // N_CORES
    n_chunks = ns // CH
    in_maps = []
    for c in range(N_CORES):
        ec = e16[c * ns:(c + 1) * ns]
        # [chunk*128 + p, kb, n] = ec[chunk*CH + n, kb*128 + p]
        ed = np.ascontiguousarray(
            ec.reshape(n_chunks, CH, 4, 128).transpose(0, 3, 2, 1)
        ).reshape(n_chunks * 128, 4, CH)
        in_maps.append({"q": qT, "e": ed})
    return in_maps, ns


def merge_candidates(per_core, ns, k):
    """Decode the packed (bf16 raw score | window-local index code)
    candidates of all cores and units, apply the 0.3 threshold mask, and
    reduce to the global top-k (value desc, index asc) — matching
    jax.lax.top_k on the masked array.

    Exactness: every score above the threshold that can enter the global
    top-5 is within its window's top-8 (a window contributes at most 5),
    so the survivor set is complete; -1 fills use the smallest free
    indices, matching top_k's tie-break on the all(-1) masked tail."""
    import ml_dtypes

    inv = 1.0 / SCALE
    n_units = ns // WIN                     # 8 windows per half
    unit_base = np.arange(n_units) * WIN
    packed = np.concatenate(
        [np.ascontiguousarray(np.asarray(r["vals8"], dtype=np.float32))
         for r in per_core], axis=1)        # [B, n_cores * n_units * 8]
    bits = packed.view(np.uint32)
    cand_vals = (bits >> 16).astype(np.uint16).view(
        ml_dtypes.bfloat16).astype(np.float32) * inv
    # window-local position; codes are window-relative (4095 - j) even for
    # the split tail units
    j_local = (WIN - 1) - (bits & 0xFFFF).astype(np.int64)
    base = np.tile(np.repeat(unit_base, 8)[None, :], (1, len(per_core)))
    core = np.repeat(np.arange(len(per_core)), n_units * 8)[None, :]
    cand_idx = core * ns + base + j_local

    surv = cand_vals > THRESH
    masked_vals = np.where(surv, cand_vals, -np.inf)
    order1 = np.argsort(cand_idx, axis=1, kind="stable")
    v1 = np.take_along_axis(masked_vals, order1, axis=1)
    i1 = np.take_along_axis(cand_idx, order1, axis=1)
    order2 = np.argsort(-v1, axis=1, kind="stable")
    vals = np.take_along_axis(v1, order2, axis=1)[:, :k].copy()
    idx = np.take_along_axis(i1, order2, axis=1)[:, :k].copy()
    nrows = vals.shape[0]
    for r in range(nrows):
        m = int((vals[r] > -np.inf).sum())
        if m >= k:
            continue
        taken = set(int(x) for x in idx[r, :m])
        fill = []
        cand = 0
        while len(fill) < k - m:
            if cand not in taken:
                fill.append(cand)
            cand += 1
        vals[r, m:] = -1.0
        idx[r, m:] = fill
    return vals.astype(np.float32), idx.astype(np.int32)


def _install_ntff_shim():
    """Register the axon NTFF profile hook (the agent image lacks
    antenv.axon_hooks; recreate it per the documented ctypes C ABI)."""
    import sys as _sys
    import types
    import ctypes
    import contextlib

    if "antenv.axon_hooks" in _sys.modules:
        return
    so_path = "/opt/axon/libaxon_pjrt.so"
    lib = ctypes.CDLL(so_path)
    if not hasattr(lib, "axon_start_nrt_profile"):
        return
    lib.axon_start_nrt_profile.argtypes = [
        ctypes.POINTER(ctypes.c_int64), ctypes.c_size_t]
    lib.axon_start_nrt_profile.restype = ctypes.c_int64
    lib.axon_stop_nrt_profile.argtypes = [ctypes.c_char_p]
    lib.axon_stop_nrt_profile.restype = ctypes.c_int64

    @contextlib.contextmanager
    def _hook(output_dir, device_ids):
        import jax
        jax.devices()
        if device_ids:
            ids = (ctypes.c_int64 * len(device_ids))(*device_ids)
            rc = lib.axon_start_nrt_profile(ids, len(device_ids))
        else:
            rc = lib.axon_start_nrt_profile(None, 0)
        if rc != 0:
            raise RuntimeError(f"axon_start_nrt_profile rc={rc}")
        try:
            yield
        finally:
            n = lib.axon_stop_nrt_profile(str(output_dir).encode())
            print(f"ntff profile: {n} file(s) written to {output_dir}",
                  file=_sys.stderr)

    mod = types.ModuleType("antenv.axon_hooks")
    mod._hook = _hook
    mod.get_axon_ntff_profile_hook = lambda: _hook
    mod.set_axon_ntff_profile_hook = lambda h: None
    _sys.modules["antenv.axon_hooks"] = mod


def kernel(query, mem_questions, mem_responses, mem_traces, mem_strengths,
           top_k, _trace=False, _results_box=None):
    from concourse import bass_utils

    if _trace:
        _install_ntff_shim()

    k = int(top_k)
    in_maps, ns = make_in_maps(
        query, mem_questions, mem_responses, mem_traces, mem_strengths)
    nc = _get_program(ns)
    res = bass_utils.run_bass_kernel_spmd(
        nc, in_maps, core_ids=list(range(N_CORES)), trace=_trace)
    if _results_box is not None:
        _results_box.append(res)
    return merge_candidates(res.results, ns, k)
